# revision 7
# baseline (speedup 1.0000x reference)
"""AdditiveAttentionPooling on 8 TRN2 NeuronCores (Bass/Tile).

Data-parallel over batch: B=32 rows -> 4 rows per core. Single streaming
pass over x: per 128-token slice, one fused VectorE scalar_tensor_tensor
computes masked scores s = sum_d x*w (accum_out), ScalarE computes
p = exp(s + b + mask_bias), TensorE accumulates p^T @ x in PSUM across the
row; the denominator sum(p) comes from one ScalarE accumulate plus a tiny
matmul per row. Normalize and store at row end. No max-subtraction needed:
softmax is shift-invariant, scores are O(1) here, and masked positions get
a -100 additive bias in the exp so they underflow to 0 exactly like the
reference's -1e9 path.
"""

import sys

sys.path.insert(0, "/opt/trn_rl_repo")

import numpy as np

import concourse.bass as bass
import concourse.tile as tile
from concourse import mybir
from concourse.bass_utils import run_bass_kernel_spmd
from concourse.vector_clock import ScopedClock

N_CORES = 8
B, T, D = 32, 2048, 1024
ROWS_PER_CORE = B // N_CORES          # 4
TOK_PER_CORE = ROWS_PER_CORE * T      # 8192
P = 128                               # SBUF partitions
TOK_PER_DMA = 1024                    # tokens per x DMA
J = TOK_PER_DMA // P                  # tokens per partition per DMA (8)
H = T // TOK_PER_DMA                  # DMAs per row (2)
SLICES_PER_ROW = T // P               # 16
MASK_NEG = -100.0                     # exp(-100) ~ 4e-44 ~ 0 in f32

F32 = mybir.dt.float32

# "f32": x stays f32 end-to-end; matmuls run as f32r (single PE pass).
# "bf16": x cast to bf16 on the host; halves HBM traffic and speeds DVE.
MODE = "bf16"

# ---------------------------------------------------------------------------
# walrus-compat patches: the walrus build in this container rejects any
# instruction carrying more than one sync-wait ("Too many sync wait
# commands"), while Tile freely attaches one wait per producer. Split the
# extras onto NoOp instructions committed just before on the same engine
# (sequential on one engine => identical semantics).
# ---------------------------------------------------------------------------

_orig_commit = tile.TileContext._commit_instruction


def _commit_split_waits(self, inst, lazy_reg_writes=True):
    si = getattr(inst, "sync_info", None)
    if si is not None and si.on_wait is not None and len(si.on_wait) > 1:
        waits = list(si.on_wait)
        si.on_wait = waits[-1:]
        nop = mybir.InstNoOp(
            name=self.nc.get_next_instruction_name(),
            engine=inst.engine,
            bass_nofuse=True,
            sync_info=mybir.SyncInfo(on_wait=waits[:-1], on_update=[]),
        )
        _commit_split_waits(self, nop, lazy_reg_writes)
    return _orig_commit(self, inst, lazy_reg_writes)


tile.TileContext._commit_instruction = _commit_split_waits


def _drain_and_barrier_split_waits(self, tick_clock, wait_clock):
    """Same single-wait constraint for the kernel-tail drain: spread its
    per-DMA-lane waits over a chain of drain instructions on SyncE."""
    nc = self.nc
    drain_inst = nc.sync.drain()
    wait_clock.add_sem_waits(
        drain_inst.ins, ScopedClock({None: tick_clock.global_clock})
    )
    waits = list(drain_inst.ins.sync_info.on_wait)
    if len(waits) > 1:
        drain_inst.ins.sync_info.on_wait = [waits[0]]
        for w in waits[1:]:
            extra = nc.sync.drain()
            extra.ins.sync_info = mybir.SyncInfo(on_wait=[w], on_update=[])
    nc.all_engine_barrier()
    popped = nc._tile_sem_poison_stack.pop()
    assert popped is self._sem_poison
    nc.clear_and_free_semaphores(list(self.sems.allocated().values()))
    nc.all_engine_barrier()


tile.TileContext._drain_and_barrier = _drain_and_barrier_split_waits

# ---------------------------------------------------------------------------


def build_graph(mode=None):
    mode = mode or MODE
    if mode == "bf16":
        DT = mybir.dt.bfloat16      # storage/matmul dtype
        VT = mybir.dt.bfloat16      # dtype DVE sees
    else:
        DT = mybir.dt.float32r      # storage/matmul dtype (same bits as f32)
        VT = mybir.dt.float32

    nc = bass.Bass()
    x = nc.declare_dram_parameter("x", [TOK_PER_CORE, D], DT, isOutput=False)
    wrep = nc.declare_dram_parameter("wrep", [P, D], VT, isOutput=False)
    # mebias[p, c] = b + (0 if mask else MASK_NEG) for slice-column c
    mebias = nc.declare_dram_parameter(
        "mebias", [P, ROWS_PER_CORE * H * J], F32, isOutput=False
    )
    out = nc.declare_dram_parameter("out", [ROWS_PER_CORE, D], F32, isOutput=True)

    with tile.TileContext(nc) as tc:
        with (
            tc.tile_pool(name="xpool", bufs=4) as xpool,
            tc.tile_pool(name="singles", bufs=1) as singles,
            tc.tile_pool(name="small", bufs=8) as small,
            tc.tile_pool(name="prow", bufs=2) as prowp,
            tc.tile_pool(name="epi", bufs=2) as epi,
            tc.tile_pool(name="psum", bufs=2, space="PSUM") as psum_pool,
        ):
            wrep_t = singles.tile([P, D], VT)
            nc.sync.dma_start(out=wrep_t, in_=wrep[:, :])
            mebias_t = singles.tile([P, ROWS_PER_CORE * H * J], F32)
            nc.sync.dma_start(out=mebias_t, in_=mebias[:, :])
            ones_t = singles.tile([P, 1], F32)
            nc.vector.memset(ones_t, 1.0)
            # stride-0 broadcast target for the fused-reduce elementwise output
            scr_t = singles.tile([P, 1], VT)

            for r in range(ROWS_PER_CORE):
                ps_n0 = psum_pool.tile([1, 512], F32)
                ps_n1 = psum_pool.tile([1, 512], F32)
                ps_den = psum_pool.tile([1, 2], F32)
                p_row = prowp.tile([P, SLICES_PER_ROW], DT)
                for h in range(H):
                    t0 = r * T + h * TOK_PER_DMA
                    xt = xpool.tile([P, J * D], DT)
                    nc.sync.dma_start(
                        out=xt,
                        in_=x[t0 : t0 + TOK_PER_DMA, :].rearrange(
                            "(p j) d -> p (j d)", p=P
                        ),
                    )
                    for j in range(J):
                        xs = xt[:, j * D : (j + 1) * D]
                        xs_v = xs if mode == "bf16" else xs.bitcast(F32)
                        c = h * J + j
                        s_t = small.tile([P, 1], F32, tag="s")
                        nc.vector.scalar_tensor_tensor(
                            out=scr_t.broadcast_to((P, D)),
                            in0=xs_v,
                            scalar=1.0,
                            in1=wrep_t,
                            op0=mybir.AluOpType.mult,
                            op1=mybir.AluOpType.mult,
                            accum_out=s_t,
                        )
                        pcol = p_row[:, c : c + 1]
                        nc.scalar.activation(
                            out=pcol,
                            in_=s_t,
                            func=mybir.ActivationFunctionType.Exp,
                            bias=mebias_t[:, (r * H + h) * J + j : (r * H + h) * J + j + 1],
                            scale=1.0,
                        )
                        first = c == 0
                        last = c == SLICES_PER_ROW - 1
                        nc.tensor.matmul(
                            ps_n0, lhsT=pcol, rhs=xs[:, 0:512], start=first, stop=last
                        )
                        nc.tensor.matmul(
                            ps_n1, lhsT=pcol, rhs=xs[:, 512:1024], start=first, stop=last
                        )
                # denominator: sum p over the row's 16 slices, then over partitions
                prow_sum = small.tile([P, 1], F32, tag="prsum")
                scr2 = small.tile([P, 1], F32, tag="scr2")
                nc.scalar.activation(
                    out=scr2.broadcast_to((P, SLICES_PER_ROW)),
                    in_=p_row,
                    func=mybir.ActivationFunctionType.Copy,
                    accum_out=prow_sum,
                )
                nc.tensor.matmul(
                    ps_den[:, 0:1], lhsT=prow_sum, rhs=ones_t, start=True, stop=True
                )
                # normalize row r and store
                num_t = epi.tile([1, D], F32)
                nc.scalar.copy(out=num_t[:, 0:512], in_=ps_n0)
                nc.scalar.copy(out=num_t[:, 512:1024], in_=ps_n1)
                rden_t = epi.tile([1, 1], F32)
                nc.vector.reciprocal(rden_t, ps_den[:, 0:1])
                o_t = epi.tile([1, D], F32)
                nc.vector.tensor_scalar_mul(out=o_t, in0=num_t, scalar1=rden_t)
                nc.sync.dma_start(out=out[r : r + 1, :], in_=o_t)

    return nc


def make_in_maps(x, mask, w, b, mode=None):
    mode = mode or MODE
    if mode == "bf16":
        import ml_dtypes

        np_dt = ml_dtypes.bfloat16
    else:
        np_dt = np.float32
    wrep = np.ascontiguousarray(
        np.broadcast_to(np.asarray(w).ravel()[None, :], (P, D))
    ).astype(np_dt)
    bval = float(np.asarray(b).ravel()[0])
    in_maps = []
    for core in range(N_CORES):
        rows = slice(core * ROWS_PER_CORE, (core + 1) * ROWS_PER_CORE)
        xc = np.ascontiguousarray(
            np.asarray(x[rows]).reshape(TOK_PER_CORE, D).astype(np_dt)
        )
        m = np.asarray(mask[rows], bool).reshape(ROWS_PER_CORE, H, P, J)
        mb = np.where(m, bval, bval + MASK_NEG).astype(np.float32)
        mb = np.ascontiguousarray(mb.transpose(2, 0, 1, 3).reshape(P, -1))
        in_maps.append({"x": xc, "wrep": wrep, "mebias": mb})
    return in_maps


def run(x, mask, w, b, trace=False, mode=None):
    nc = build_graph(mode)
    in_maps = make_in_maps(x, mask, w, b, mode)
    res = run_bass_kernel_spmd(nc, in_maps, core_ids=list(range(N_CORES)), trace=trace)
    out = np.concatenate([res.results[i]["out"] for i in range(N_CORES)], axis=0)
    return out, res


def kernel(x, mask, w, b):
    out, _ = run(x, mask, w, b, trace=False)
    return out


# revision 10
# speedup vs baseline: 1.1561x; 1.1561x over previous
"""AdditiveAttentionPooling on 8 TRN2 NeuronCores (Bass/Tile).

Data-parallel over batch: B=32 rows -> 4 rows per core. Single streaming
pass over x: per 128-token slice, one fused VectorE scalar_tensor_tensor
computes masked scores s = sum_d x*w (accum_out), ScalarE computes
p = exp(s + b + mask_bias), TensorE accumulates p^T @ x in PSUM across the
row; the denominator sum(p) comes from one ScalarE accumulate plus a tiny
matmul per row. Normalize and store at row end. No max-subtraction needed:
softmax is shift-invariant, scores are O(1) here, and masked positions get
a -100 additive bias in the exp so they underflow to 0 exactly like the
reference's -1e9 path.
"""

import sys

sys.path.insert(0, "/opt/trn_rl_repo")

import numpy as np

import concourse.bass as bass
import concourse.tile as tile
from concourse import mybir
from concourse.bass_utils import run_bass_kernel_spmd
from concourse.vector_clock import ScopedClock

N_CORES = 8
B, T, D = 32, 2048, 1024
ROWS_PER_CORE = B // N_CORES          # 4
TOK_PER_CORE = ROWS_PER_CORE * T      # 8192
P = 128                               # SBUF partitions
TOK_PER_DMA = 1024                    # tokens per x DMA
J = TOK_PER_DMA // P                  # tokens per partition per DMA (8)
H = T // TOK_PER_DMA                  # DMAs per row (2)
SLICES_PER_ROW = T // P               # 16
MASK_NEG = -100.0                     # exp(-100) ~ 4e-44 ~ 0 in f32

F32 = mybir.dt.float32

# "f32": x stays f32 end-to-end; matmuls run as f32r (single PE pass).
# "bf16": x cast to bf16 on the host; halves HBM traffic and speeds DVE.
MODE = "bf16"

# ---------------------------------------------------------------------------
# walrus-compat patches: the walrus build in this container rejects any
# instruction carrying more than one sync-wait ("Too many sync wait
# commands"), while Tile freely attaches one wait per producer. Split the
# extras onto NoOp instructions committed just before on the same engine
# (sequential on one engine => identical semantics).
# ---------------------------------------------------------------------------

_orig_commit = tile.TileContext._commit_instruction


def _commit_split_waits(self, inst, lazy_reg_writes=True):
    si = getattr(inst, "sync_info", None)
    if si is not None and si.on_wait is not None and len(si.on_wait) > 1:
        waits = list(si.on_wait)
        si.on_wait = waits[-1:]
        nop = mybir.InstNoOp(
            name=self.nc.get_next_instruction_name(),
            engine=inst.engine,
            bass_nofuse=True,
            sync_info=mybir.SyncInfo(on_wait=waits[:-1], on_update=[]),
        )
        _commit_split_waits(self, nop, lazy_reg_writes)
    return _orig_commit(self, inst, lazy_reg_writes)


tile.TileContext._commit_instruction = _commit_split_waits


def _drain_and_barrier_split_waits(self, tick_clock, wait_clock):
    """Same single-wait constraint for the kernel-tail drain: spread its
    per-DMA-lane waits over a chain of drain instructions on SyncE."""
    nc = self.nc
    drain_inst = nc.sync.drain()
    wait_clock.add_sem_waits(
        drain_inst.ins, ScopedClock({None: tick_clock.global_clock})
    )
    waits = list(drain_inst.ins.sync_info.on_wait)
    if len(waits) > 1:
        drain_inst.ins.sync_info.on_wait = [waits[0]]
        for w in waits[1:]:
            extra = nc.sync.drain()
            extra.ins.sync_info = mybir.SyncInfo(on_wait=[w], on_update=[])
    nc.all_engine_barrier()
    popped = nc._tile_sem_poison_stack.pop()
    assert popped is self._sem_poison
    nc.clear_and_free_semaphores(list(self.sems.allocated().values()))
    nc.all_engine_barrier()


tile.TileContext._drain_and_barrier = _drain_and_barrier_split_waits

# ---------------------------------------------------------------------------


def build_graph(mode=None):
    mode = mode or MODE
    if mode == "bf16":
        DT = mybir.dt.bfloat16      # storage/matmul dtype
        VT = mybir.dt.bfloat16      # dtype DVE sees
    else:
        DT = mybir.dt.float32r      # storage/matmul dtype (same bits as f32)
        VT = mybir.dt.float32

    nc = bass.Bass()
    x = nc.declare_dram_parameter("x", [TOK_PER_CORE, D], DT, isOutput=False)
    wrep = nc.declare_dram_parameter("wrep", [P, D], VT, isOutput=False)
    # mebias[p, c] = b + (0 if mask else MASK_NEG) for slice-column c
    mebias = nc.declare_dram_parameter(
        "mebias", [P, ROWS_PER_CORE * H * J], F32, isOutput=False
    )
    out = nc.declare_dram_parameter("out", [ROWS_PER_CORE, D], F32, isOutput=True)

    with tile.TileContext(nc) as tc:
        with (
            tc.tile_pool(name="xpool", bufs=4) as xpool,
            tc.tile_pool(name="singles", bufs=1) as singles,
            tc.tile_pool(name="small", bufs=8) as small,
            tc.tile_pool(name="prow", bufs=2) as prowp,
            tc.tile_pool(name="epi", bufs=2) as epi,
            tc.tile_pool(name="psum", bufs=2, space="PSUM") as psum_pool,
        ):
            wrep_t = singles.tile([P, D], VT)
            nc.sync.dma_start(out=wrep_t, in_=wrep[:, :])
            mebias_t = singles.tile([P, ROWS_PER_CORE * H * J], F32)
            nc.sync.dma_start(out=mebias_t, in_=mebias[:, :])
            ones_t = singles.tile([P, 1], F32)
            nc.vector.memset(ones_t, 1.0)
            # stride-0 broadcast target for the fused-reduce elementwise output
            scr_t = singles.tile([P, 1], VT)
            # real (step-1) scratch for the fused-reduce output: a stride-0
            # broadcast out disqualifies the DVE 2x packed mode for 16-bit
            if mode == "bf16":
                scr_full = singles.tile([P, D], VT, tag="scr_full")
            else:
                scr_full = None

            for r in range(ROWS_PER_CORE):
                ps_n0 = psum_pool.tile([1, 512], F32)
                ps_n1 = psum_pool.tile([1, 512], F32)
                ps_den = psum_pool.tile([1, 2], F32)
                p_row = prowp.tile([P, SLICES_PER_ROW], DT)
                for h in range(H):
                    t0 = r * T + h * TOK_PER_DMA
                    xt = xpool.tile([P, J * D], DT)
                    nc.sync.dma_start(
                        out=xt,
                        in_=x[t0 : t0 + TOK_PER_DMA, :].rearrange(
                            "(p j) d -> p (j d)", p=P
                        ),
                    )
                    for j in range(J):
                        xs = xt[:, j * D : (j + 1) * D]
                        xs_v = xs if mode == "bf16" else xs.bitcast(F32)
                        c = h * J + j
                        s_t = small.tile([P, 1], F32, tag="s")
                        nc.vector.scalar_tensor_tensor(
                            out=scr_full if scr_full is not None
                            else scr_t.broadcast_to((P, D)),
                            in0=xs_v,
                            scalar=1.0,
                            in1=wrep_t,
                            op0=mybir.AluOpType.mult,
                            op1=mybir.AluOpType.mult,
                            accum_out=s_t,
                        )
                        pcol = p_row[:, c : c + 1]
                        nc.scalar.activation(
                            out=pcol,
                            in_=s_t,
                            func=mybir.ActivationFunctionType.Exp,
                            bias=mebias_t[:, (r * H + h) * J + j : (r * H + h) * J + j + 1],
                            scale=1.0,
                        )
                        first = c == 0
                        last = c == SLICES_PER_ROW - 1
                        nc.tensor.matmul(
                            ps_n0, lhsT=pcol, rhs=xs[:, 0:512], start=first, stop=last
                        )
                        nc.tensor.matmul(
                            ps_n1, lhsT=pcol, rhs=xs[:, 512:1024], start=first, stop=last
                        )
                # denominator: sum p over the row's 16 slices, then over partitions
                prow_sum = small.tile([P, 1], F32, tag="prsum")
                scr2 = small.tile([P, 1], F32, tag="scr2")
                nc.scalar.activation(
                    out=scr2.broadcast_to((P, SLICES_PER_ROW)),
                    in_=p_row,
                    func=mybir.ActivationFunctionType.Copy,
                    accum_out=prow_sum,
                )
                nc.tensor.matmul(
                    ps_den[:, 0:1], lhsT=prow_sum, rhs=ones_t, start=True, stop=True
                )
                # normalize row r and store
                num_t = epi.tile([1, D], F32)
                nc.scalar.copy(out=num_t[:, 0:512], in_=ps_n0)
                nc.scalar.copy(out=num_t[:, 512:1024], in_=ps_n1)
                rden_t = epi.tile([1, 1], F32)
                nc.vector.reciprocal(rden_t, ps_den[:, 0:1])
                o_t = epi.tile([1, D], F32)
                nc.vector.tensor_scalar_mul(out=o_t, in0=num_t, scalar1=rden_t)
                nc.sync.dma_start(out=out[r : r + 1, :], in_=o_t)

    return nc


def make_in_maps(x, mask, w, b, mode=None):
    mode = mode or MODE
    if mode == "bf16":
        import ml_dtypes

        np_dt = ml_dtypes.bfloat16
    else:
        np_dt = np.float32
    wrep = np.ascontiguousarray(
        np.broadcast_to(np.asarray(w).ravel()[None, :], (P, D))
    ).astype(np_dt)
    bval = float(np.asarray(b).ravel()[0])
    in_maps = []
    for core in range(N_CORES):
        rows = slice(core * ROWS_PER_CORE, (core + 1) * ROWS_PER_CORE)
        xc = np.ascontiguousarray(
            np.asarray(x[rows]).reshape(TOK_PER_CORE, D).astype(np_dt)
        )
        m = np.asarray(mask[rows], bool).reshape(ROWS_PER_CORE, H, P, J)
        mb = np.where(m, bval, bval + MASK_NEG).astype(np.float32)
        mb = np.ascontiguousarray(mb.transpose(2, 0, 1, 3).reshape(P, -1))
        in_maps.append({"x": xc, "wrep": wrep, "mebias": mb})
    return in_maps


def run(x, mask, w, b, trace=False, mode=None):
    nc = build_graph(mode)
    in_maps = make_in_maps(x, mask, w, b, mode)
    res = run_bass_kernel_spmd(nc, in_maps, core_ids=list(range(N_CORES)), trace=trace)
    out = np.concatenate([res.results[i]["out"] for i in range(N_CORES)], axis=0)
    return out, res


def kernel(x, mask, w, b):
    out, _ = run(x, mask, w, b, trace=False)
    return out
